# revision 19
# baseline (speedup 1.0000x reference)
"""GAT-VGAE forward pass on 8 Trainium2 NeuronCores (Bass/Tile).

Strategy
--------
- Edges are bucketed by destination node range on the host: core c owns dst
  nodes [256c, 256c+256).  Segment-softmax over incoming edges never needs a
  segment max: the logits of this problem are bounded (|logit| < ~6), so
  exp(logit) is computed directly (softmax is shift-invariant).
- Per-edge gathers use batched SWDGE dma_gather (1280 indices per call) from
  "augmented" row tables (h | a_src | a_dst), edges landing on partitions.
- Segment sums (denominators + weighted message aggregation) are one-hot
  matmuls accumulated in PSUM: lhsT = onehot(dst_local) [128e x 256d],
  rhs = payload [128e x F].  One-hots are built once and reused by layer 2.
- Matmul operands are bf16 (PSUM accumulates fp32); attention arithmetic
  (logits, exp, normalization) stays fp32 on DVE/ACT.
- Layer-1 output (hidden) is transposed on-device and AllGathered (bf16) so
  each core can form lhsT tiles of hidden for the layer-2 matmul.
- z-mean is a ones-matmul partition reduction + tiny AllReduce.
- The huge decoder weight Wd [64, N*N] (1 GiB) is sharded column-wise:
  67 MB/core in bf16, pre-arranged on the host into [128,128] lhsT tiles
  packing two 128-column chunks along K (rhs = [[zm,0],[0,zm]]), so each
  matmul streams 32 KB of Wd and lands 256 outputs on 128 partitions.
  Decoder weight DMA rides the ACT HWDGE ring so it cannot head-of-line
  block the phase-critical loads on the sync ring.  Sigmoid is applied by
  ScalarE straight out of PSUM bank fills.
"""
import sys

sys.path.insert(0, '/opt/trn_rl_repo')

import ml_dtypes
import numpy as np

import bass_rust
import concourse.bass as bass
import concourse.bacc as bacc
import concourse.mybir as mybir
import concourse.tile as tile
from concourse import library_config
from concourse.bass import IndirectOffsetOnAxis
from concourse.bass_utils import run_bass_kernel_spmd
from concourse.masks import make_identity
from concourse.tile import add_dep_helper

F32 = mybir.dt.float32
BF16 = mybir.dt.bfloat16
I16 = mybir.dt.int16
AF = mybir.ActivationFunctionType
OP = mybir.AluOpType

N = 2048
F_IN = 256
C1 = 128
H = 4
HID = H * C1          # 512
EMB = 64
NCORES = 8
DPC = N // NCORES     # 256 dst nodes per core
COLS = N * N // NCORES  # 524288 decoder columns per core
NEG = 0.2
P = 128
H1ROW = 576           # h1(512) | a_src1(4) | a_dst1(4) | pad -> 2304B rows
H2ROW = 128           # h2(64) | a_src2(1) | a_dst2(1) | pad -> 512B rows
DROW = 64             # dst-table rows: 256B
GB = 8                # edge tiles per dma_gather call (1024 idxs; >1024 crashes SWDGE)
WD_GROUP = 32         # decoder lhsT tiles per DMA group
WD_NGROUPS = COLS // (256 * WD_GROUP)  # 64
RG = [list(range(NCORES))]

_MAX_WAITS = 1
_wait_ctr = [0]


def _split_excess_waits(nc):
    """This container's walrus accepts only one sync-wait per instruction.
    Hoist excess waits onto InstNoOps inserted just before, same engine."""
    for f in nc.m.functions:
        for blk in f.blocks:
            out = []
            changed = False
            for inst in blk.instructions:
                si = inst.sync_info
                waits = list(si.on_wait) if si is not None else []
                if len(waits) > _MAX_WAITS:
                    changed = True
                    extra, keep = waits[:-_MAX_WAITS], waits[-_MAX_WAITS:]
                    for i in range(0, len(extra), _MAX_WAITS):
                        nop = bass_rust.InstNoOp(
                            name=f"waitsplit-{_wait_ctr[0]}", ins=[], outs=[])
                        _wait_ctr[0] += 1
                        nop.engine = inst.engine
                        nop.sync_info = bass_rust.SyncInfo(
                            on_wait=extra[i:i + _MAX_WAITS], on_update=[])
                        out.append(nop)
                    inst.sync_info = bass_rust.SyncInfo(
                        on_wait=keep, on_update=list(si.on_update))
                out.append(inst)
            if changed:
                blk.instructions = out


def _leaky(nc, sb, x_ap, w):
    """leaky_relu(x) = max(x, NEG*x) on DVE (ACT Lrelu ignores alpha)."""
    t = sb.tile([P, w], F32)
    nc.vector.tensor_scalar_mul(t[:], x_ap, NEG)
    nc.vector.tensor_tensor(out=t[:], in0=t[:], in1=x_ap, op=OP.max)
    return t


def build_program(T):
    """T = number of 128-edge tiles per core (multiple of GB)."""
    assert T % GB == 0
    ncall = T // GB
    icols = GB * P // 16  # idx columns per gather call (64)
    nc = bacc.Bacc("TRN2", num_devices=NCORES)

    # ---- I/O -------------------------------------------------------------
    xT_d = nc.dram_tensor("xT", [F_IN, N], BF16, kind="ExternalInput")
    w1_d = nc.dram_tensor("W1", [F_IN, HID], BF16, kind="ExternalInput")
    w2_d = nc.dram_tensor("W2", [HID, EMB], BF16, kind="ExternalInput")
    wmu_d = nc.dram_tensor("Wmu", [EMB, EMB], BF16, kind="ExternalInput")
    wlv_d = nc.dram_tensor("Wlv", [EMB, EMB], BF16, kind="ExternalInput")
    asd1_d = nc.dram_tensor("asd1r", [P, 2 * HID], F32, kind="ExternalInput")
    b1_d = nc.dram_tensor("b1r", [P, HID], F32, kind="ExternalInput")
    as2_d = nc.dram_tensor("as2r", [P, EMB], F32, kind="ExternalInput")
    ad2_d = nc.dram_tensor("ad2r", [P, EMB], F32, kind="ExternalInput")
    b2_d = nc.dram_tensor("b2r", [P, EMB], F32, kind="ExternalInput")
    bmu_d = nc.dram_tensor("bmur", [P, EMB], F32, kind="ExternalInput")
    blv_d = nc.dram_tensor("blvr", [P, EMB], F32, kind="ExternalInput")
    eps_d = nc.dram_tensor("epsl", [DPC, EMB], F32, kind="ExternalInput")
    esrc16_d = nc.dram_tensor("esrc16", [P, ncall * icols], I16,
                              kind="ExternalInput")
    edstg16_d = nc.dram_tensor("edstg16", [P, ncall * icols], I16,
                               kind="ExternalInput")
    edstl_d = nc.dram_tensor("edstl", [P, T], F32, kind="ExternalInput")
    wd_d = nc.dram_tensor("wd", [WD_NGROUPS, P, WD_GROUP * P], BF16,
                          kind="ExternalInput")
    bd_d = nc.dram_tensor("bd", [8, P, 512], F32, kind="ExternalInput")
    out_d = nc.dram_tensor("out", [8, P, 512], F32, kind="ExternalOutput")

    # ---- internal DRAM gather tables -------------------------------------
    h1aug_d = nc.dram_tensor("h1aug", [N, H1ROW], F32, kind="Internal")
    daug1_d = nc.dram_tensor("daug1", [N, DROW], F32, kind="Internal")
    dlocal2_d = nc.dram_tensor("dlocal2", [DPC, 1], F32, kind="Internal")

    with tile.TileContext(nc) as tc:
        with (
            tc.tile_pool(name="consts", bufs=1) as consts,
            tc.tile_pool(name="dram", bufs=1, space="DRAM") as dram,
            tc.tile_pool(name="sb", bufs=3) as sb,
        ):
            # ---- constants ------------------------------------------------
            iota_i = consts.tile([P, 2 * P], mybir.dt.int32)
            iota_inst = nc.gpsimd.iota(iota_i[:], pattern=[[1, 2 * P]], base=0,
                                       channel_multiplier=0)
            iota_f = consts.tile([P, 2 * P], F32)
            nc.vector.tensor_copy(iota_f[:], iota_i[:])
            lib_inst = nc.gpsimd.load_library(library_config.mlp)
            add_dep_helper(lib_inst.ins, iota_inst.ins, sync=True,
                           reason="iota (standard lib) before mlp lib load")
            ident = consts.tile([P, P], F32)
            make_identity(nc, ident[:])
            ones = consts.tile([P, 1], F32)
            nc.vector.memset(ones[:], 1.0)

            xt_sb = [consts.tile([P, N], BF16, tag=f"xt{i}", name=f"xt{i}")
                     for i in range(2)]
            for i in range(2):
                nc.sync.dma_start(xt_sb[i][:], xT_d[i * P:(i + 1) * P, :])
            w1_sb = [consts.tile([P, HID], BF16, tag=f"w1{i}", name=f"w1s{i}")
                     for i in range(2)]
            for i in range(2):
                nc.sync.dma_start(w1_sb[i][:], w1_d[i * P:(i + 1) * P, :])
            w2_sb = [consts.tile([P, EMB], BF16, tag=f"w2{i}", name=f"w2s{i}")
                     for i in range(4)]
            for i in range(4):
                nc.sync.dma_start(w2_sb[i][:], w2_d[i * P:(i + 1) * P, :])
            wmu_sb = consts.tile([EMB, EMB], BF16)
            nc.sync.dma_start(wmu_sb[:], wmu_d[:])
            wlv_sb = consts.tile([EMB, EMB], BF16)
            nc.sync.dma_start(wlv_sb[:], wlv_d[:])
            asd1_sb = consts.tile([P, 2 * HID], F32)
            nc.scalar.dma_start(asd1_sb[:], asd1_d[:])
            b1_sb = consts.tile([P, HID], F32)
            nc.scalar.dma_start(b1_sb[:], b1_d[:])
            as2_sb = consts.tile([P, EMB], F32)
            nc.scalar.dma_start(as2_sb[:], as2_d[:])
            ad2_sb = consts.tile([P, EMB], F32)
            nc.scalar.dma_start(ad2_sb[:], ad2_d[:])
            b2_sb = consts.tile([P, EMB], F32)
            nc.scalar.dma_start(b2_sb[:], b2_d[:])
            bmu_sb = consts.tile([P, EMB], F32)
            nc.scalar.dma_start(bmu_sb[:], bmu_d[:])
            blv_sb = consts.tile([P, EMB], F32)
            nc.scalar.dma_start(blv_sb[:], blv_d[:])
            eps_sb = [consts.tile([P, EMB], F32, tag=f"eps{i}", name=f"epss{i}")
                      for i in range(2)]
            for i in range(2):
                nc.sync.dma_start(eps_sb[i][:], eps_d[i * P:(i + 1) * P, :])
            esrc16_sb = consts.tile([P, ncall * icols], I16)
            nc.sync.dma_start(esrc16_sb[:], esrc16_d[:])
            edstg16_sb = consts.tile([P, ncall * icols], I16)
            nc.sync.dma_start(edstg16_sb[:], edstg16_d[:])
            edstl_sb = consts.tile([P, T], F32)
            nc.sync.dma_start(edstl_sb[:], edstl_d[:])
            # one-hots built in phase 1, reused in phase 2b
            ohall = consts.tile([P, T * 2 * P], BF16)

            def gather(table, idx_sb, c, width, tag, bufs=2):
                g = sb.tile([P, GB, width], F32, tag=tag, name=f"{tag}{c}",
                            bufs=bufs)
                gi = nc.gpsimd.dma_gather(
                    g[:], table[:, :], idx_sb[:, c * icols:(c + 1) * icols],
                    GB * P, GB * P, width)
                add_dep_helper(gi.ins, lib_inst.ins, sync=True,
                               reason="dma_gather needs mlp library")
                return g

            # ---- phase 0: h1 = x @ W1, a_src1/a_dst1, build gather tables -
            with tc.tile_pool(name="ps0", bufs=2, space="PSUM") as ps0:
                for m in range(N // P):
                    ph1 = ps0.tile([P, HID], F32, space="PSUM")
                    for ck in range(2):
                        nc.tensor.matmul(
                            out=ph1[:], lhsT=xt_sb[ck][:, m * P:(m + 1) * P],
                            rhs=w1_sb[ck][:], start=(ck == 0), stop=(ck == 1))
                    aug = sb.tile([P, 520], F32, tag="h1aug", bufs=2)
                    nc.scalar.copy(aug[:, 0:HID], ph1[:])
                    tmp = sb.tile([P, 2 * HID], F32, tag="p0tmp", bufs=2)
                    nc.vector.tensor_tensor(
                        out=tmp[:].rearrange("p (s h c) -> p s h c", s=2, h=H),
                        in0=aug[:, 0:HID].rearrange(
                            "p (h c) -> p h c", h=H)[:, None, :, :]
                            .to_broadcast([P, 2, H, C1]),
                        in1=asd1_sb[:].rearrange("p (s h c) -> p s h c",
                                                 s=2, h=H),
                        op=OP.mult)
                    nc.vector.tensor_reduce(
                        out=aug[:, HID:520],
                        in_=tmp[:].rearrange("p (s h c) -> p (s h) c", s=2,
                                             h=H),
                        axis=mybir.AxisListType.X, op=OP.add)
                    nc.sync.dma_start(h1aug_d[m * P:(m + 1) * P, 0:520], aug[:])
                    nc.sync.dma_start(daug1_d[m * P:(m + 1) * P, 0:H],
                                      aug[:, HID + H:520])

            # ---- phase 1: layer-1 edge pass -------------------------------
            hidT_sb = [consts.tile([P, 2 * P], BF16, tag=f"hidT{i}",
                                   name=f"hidT{i}") for i in range(4)]
            with tc.tile_pool(name="ps1", bufs=1, space="PSUM") as ps1:
                pd1 = [ps1.tile([P, HID], F32, space="PSUM", tag=f"pd1{i}",
                                name=f"pd1{i}") for i in range(2)]
                pden = [ps1.tile([P, H], F32, space="PSUM", tag=f"pden{i}",
                                 name=f"pden{i}") for i in range(2)]
                for c in range(ncall):
                    ehg = gather(h1aug_d, esrc16_sb, c, H1ROW, "ehg")
                    eadg = gather(daug1_d, edstg16_sb, c, DROW, "eadg")
                    # batched per-call edge math: one DVE/ACT op per stage
                    lg = sb.tile([P, GB, H], F32, tag="lg1")
                    nc.vector.tensor_tensor(
                        out=lg[:], in0=ehg[:, :, HID:HID + H],
                        in1=eadg[:, :, 0:H], op=OP.add)
                    lr = sb.tile([P, GB, H], F32, tag="lr1")
                    nc.vector.tensor_scalar_mul(lr[:], lg[:], NEG)
                    nc.vector.tensor_tensor(out=lr[:], in0=lr[:], in1=lg[:],
                                            op=OP.max)
                    v = sb.tile([P, GB, H], F32, tag="v1")
                    nc.scalar.activation(v[:], lr[:], AF.Exp)
                    vb = sb.tile([P, GB, H], BF16, tag="vb1")
                    nc.vector.tensor_copy(vb[:], v[:])
                    pay = sb.tile([P, GB, HID], BF16, tag="pay1", bufs=2)
                    nc.vector.tensor_tensor(
                        out=pay[:].rearrange("p g (h c) -> p g h c", h=H),
                        in0=ehg[:, :, 0:HID].rearrange(
                            "p g (h c) -> p g h c", h=H),
                        in1=v[:, :, :, None].to_broadcast([P, GB, H, C1]),
                        op=OP.mult)
                    ohc = ohall[:, c * GB * 2 * P:(c + 1) * GB * 2 * P]
                    nc.vector.tensor_tensor(
                        out=ohc.rearrange("p (g j) -> p g j", g=GB),
                        in0=edstl_sb[:, c * GB:(c + 1) * GB, None]
                            .to_broadcast([P, GB, 2 * P]),
                        in1=iota_f[:, None, :].to_broadcast([P, GB, 2 * P]),
                        op=OP.is_equal)
                    for u in range(GB):
                        t = c * GB + u
                        oh = ohall[:, t * 2 * P:(t + 1) * 2 * P]
                        st, sp = (t == 0), (t == T - 1)
                        for half in range(2):
                            ohh = oh[:, half * P:(half + 1) * P]
                            nc.tensor.matmul(out=pd1[half][:], lhsT=ohh,
                                             rhs=pay[:, u, :], start=st, stop=sp)
                            nc.tensor.matmul(out=pden[half][:], lhsT=ohh,
                                             rhs=vb[:, u, :], start=st, stop=sp)

                # normalize + bias + relu + transpose
                recip = sb.tile([P, 2 * H], F32, tag="recip1")
                for half in range(2):
                    nc.vector.tensor_scalar_add(
                        recip[:, half * H:(half + 1) * H], pden[half][:], 1e-16)
                nc.vector.reciprocal(recip[:], recip[:])
                with tc.tile_pool(name="psT", bufs=2, space="PSUM") as psT:
                    for half in range(2):
                        agg = sb.tile([P, HID], F32, tag="agg1", bufs=2)
                        nc.scalar.copy(agg[:], pd1[half][:])
                        hid = sb.tile([P, HID], F32, tag="hid", bufs=2)
                        nc.vector.tensor_tensor(
                            out=hid[:].rearrange("p (h c) -> p h c", h=H),
                            in0=agg[:].rearrange("p (h c) -> p h c", h=H),
                            in1=recip[:, half * H:(half + 1) * H]
                                .to_broadcast([P, H, C1]),
                            op=OP.mult)
                        nc.vector.tensor_add(hid[:], hid[:], b1_sb[:])
                        nc.scalar.activation(hid[:], hid[:], AF.Relu)
                        for ck in range(4):
                            pt = psT.tile([P, P], F32, space="PSUM", tag="pt")
                            nc.tensor.transpose(
                                out=pt[:], in_=hid[:, ck * P:(ck + 1) * P],
                                identity=ident[:])
                            nc.vector.tensor_copy(
                                hidT_sb[ck][:, half * P:(half + 1) * P], pt[:])

            # ---- phase 2a: local h2 from local hidden, AllGather the table
            h2loc = dram.tile([DPC, H2ROW], F32)
            h2full = dram.tile([N, H2ROW], F32)
            with (
                tc.tile_pool(name="ps2a", bufs=1, space="PSUM") as ps2a,
                tc.tile_pool(name="ps2t", bufs=2, space="PSUM") as ps2t,
            ):
                ph2t = ps2a.tile([EMB, 2 * P], F32, space="PSUM", tag="ph2t")
                for ck in range(4):
                    nc.tensor.matmul(out=ph2t[:], lhsT=w2_sb[ck][:],
                                     rhs=hidT_sb[ck][:],
                                     start=(ck == 0), stop=(ck == 3))
                h2t_sb = sb.tile([EMB, 2 * P], F32, tag="h2ts")
                nc.vector.tensor_copy(h2t_sb[:], ph2t[:])
                for half in range(2):
                    pt = ps2t.tile([P, EMB], F32, space="PSUM", tag="p2t")
                    nc.tensor.transpose(
                        out=pt[:], in_=h2t_sb[:, half * P:(half + 1) * P],
                        identity=ident[0:EMB, 0:EMB])
                    aug2 = sb.tile([P, EMB + 2], F32, tag="h2aug")
                    nc.scalar.copy(aug2[:, 0:EMB], pt[:])
                    tmp2 = sb.tile([P, EMB], F32, tag="p2tmp")
                    nc.vector.tensor_tensor(out=tmp2[:], in0=pt[:],
                                            in1=as2_sb[:], op=OP.mult)
                    nc.vector.tensor_reduce(out=aug2[:, EMB:EMB + 1],
                                            in_=tmp2[:],
                                            axis=mybir.AxisListType.X,
                                            op=OP.add)
                    nc.vector.tensor_tensor(out=tmp2[:], in0=pt[:],
                                            in1=ad2_sb[:], op=OP.mult)
                    nc.vector.tensor_reduce(out=aug2[:, EMB + 1:EMB + 2],
                                            in_=tmp2[:],
                                            axis=mybir.AxisListType.X,
                                            op=OP.add)
                    nc.sync.dma_start(
                        h2loc[half * P:(half + 1) * P, 0:EMB + 2], aug2[:])
                    nc.sync.dma_start(dlocal2_d[half * P:(half + 1) * P, :],
                                      aug2[:, EMB + 1:EMB + 2])
            nc.gpsimd.collective_compute(
                "AllGather", OP.bypass, replica_groups=RG,
                ins=[h2loc.opt()], outs=[h2full.opt()])

            # replicate local a_dst2 across partitions for the DVE expansion
            adst2_rep = consts.tile([P, DPC], F32)
            nc.sync.dma_start(
                adst2_rep[:],
                dlocal2_d[:, :].rearrange("a b -> b a").to_broadcast(
                    [P, DPC]))

            # ---- phase 2b: layer-2 edge pass ------------------------------
            embT_sb = consts.tile([EMB, 2 * P], BF16)
            with tc.tile_pool(name="ps2b", bufs=1, space="PSUM") as ps2b:
                pd2 = [ps2b.tile([P, EMB], F32, space="PSUM", tag=f"pd2{i}",
                                 name=f"pd2{i}") for i in range(2)]
                pden2 = [ps2b.tile([P, 1], F32, space="PSUM", tag=f"pden2{i}",
                                   name=f"pden2{i}") for i in range(2)]
                for c in range(ncall):
                    eh2g = gather(h2full, esrc16_sb, c, H2ROW, "eh2g")
                    ohc = ohall[:, c * GB * 2 * P:(c + 1) * GB * 2 * P]
                    adx = sb.tile([P, GB, 2 * P], F32, tag="adx", bufs=2)
                    nc.vector.tensor_tensor(
                        out=adx[:],
                        in0=ohc.rearrange("p (g j) -> p g j", g=GB),
                        in1=adst2_rep[:, None, :].to_broadcast([P, GB, 2 * P]),
                        op=OP.mult)
                    ead2 = sb.tile([P, GB, 1], F32, tag="ead2")
                    nc.vector.tensor_reduce(out=ead2[:], in_=adx[:],
                                            axis=mybir.AxisListType.X,
                                            op=OP.add)
                    lg2 = sb.tile([P, GB, 1], F32, tag="lg2")
                    nc.vector.tensor_tensor(
                        out=lg2[:], in0=eh2g[:, :, EMB:EMB + 1],
                        in1=ead2[:], op=OP.add)
                    lr2 = sb.tile([P, GB, 1], F32, tag="lr2")
                    nc.vector.tensor_scalar_mul(lr2[:], lg2[:], NEG)
                    nc.vector.tensor_tensor(out=lr2[:], in0=lr2[:], in1=lg2[:],
                                            op=OP.max)
                    v2 = sb.tile([P, GB, 1], F32, tag="v2")
                    nc.scalar.activation(v2[:], lr2[:], AF.Exp)
                    v2b = sb.tile([P, GB, 1], BF16, tag="v2b")
                    nc.vector.tensor_copy(v2b[:], v2[:])
                    pay2 = sb.tile([P, GB, EMB], BF16, tag="pay2")
                    nc.vector.tensor_tensor(
                        out=pay2[:], in0=eh2g[:, :, 0:EMB],
                        in1=v2[:].to_broadcast([P, GB, EMB]), op=OP.mult)
                    for u in range(GB):
                        t = c * GB + u
                        oh = ohall[:, t * 2 * P:(t + 1) * 2 * P]
                        st, sp = (t == 0), (t == T - 1)
                        for half in range(2):
                            ohh = oh[:, half * P:(half + 1) * P]
                            nc.tensor.matmul(out=pd2[half][:], lhsT=ohh,
                                             rhs=pay2[:, u, :], start=st, stop=sp)
                            nc.tensor.matmul(out=pden2[half][:], lhsT=ohh,
                                             rhs=v2b[:, u, :], start=st, stop=sp)

                recip2 = sb.tile([P, 2], F32, tag="recip2")
                for half in range(2):
                    nc.vector.tensor_scalar_add(
                        recip2[:, half:half + 1], pden2[half][:], 1e-16)
                nc.vector.reciprocal(recip2[:], recip2[:])
                with tc.tile_pool(name="psT2", bufs=2, space="PSUM") as psT2:
                    for half in range(2):
                        agg2 = sb.tile([P, EMB], F32, tag="agg2", bufs=2)
                        nc.scalar.copy(agg2[:], pd2[half][:])
                        emb = sb.tile([P, EMB], F32, tag="emb")
                        nc.vector.tensor_tensor(
                            out=emb[:], in0=agg2[:],
                            in1=recip2[:, half:half + 1].to_broadcast([P, EMB]),
                            op=OP.mult)
                        nc.vector.tensor_add(emb[:], emb[:], b2_sb[:])
                        pt2 = psT2.tile([EMB, P], F32, space="PSUM", tag="pt2")
                        nc.tensor.transpose(out=pt2[:], in_=emb[:],
                                            identity=ident[:])
                        nc.vector.tensor_copy(
                            embT_sb[:, half * P:(half + 1) * P], pt2[:])

            # ---- phase 3: mu / logvar / z / z-sum -------------------------
            zs_in = dram.tile([EMB, 1], F32)
            zs_out = dram.tile([EMB, 1], F32)
            with tc.tile_pool(name="ps3", bufs=1, space="PSUM") as ps3:
                pzs = ps3.tile([EMB, 1], F32, space="PSUM", tag="pzs")
                for half in range(2):
                    lhs = embT_sb[:, half * P:(half + 1) * P]
                    pmu = ps3.tile([P, EMB], F32, space="PSUM",
                                   tag=f"pmu{half}", name=f"pmu{half}")
                    nc.tensor.matmul(out=pmu[:], lhsT=lhs, rhs=wmu_sb[:],
                                     start=True, stop=True)
                    plv = ps3.tile([P, EMB], F32, space="PSUM",
                                   tag=f"plv{half}", name=f"plv{half}")
                    nc.tensor.matmul(out=plv[:], lhsT=lhs, rhs=wlv_sb[:],
                                     start=True, stop=True)
                    elv = sb.tile([P, EMB], F32, tag="elv")
                    nc.vector.tensor_add(elv[:], plv[:], blv_sb[:])
                    nc.scalar.activation(elv[:], elv[:], AF.Exp, scale=0.5)
                    z = sb.tile([P, EMB], F32, tag="z")
                    nc.vector.tensor_tensor(out=z[:], in0=elv[:],
                                            in1=eps_sb[half][:], op=OP.mult)
                    nc.vector.tensor_add(z[:], z[:], bmu_sb[:])
                    nc.vector.tensor_add(z[:], z[:], pmu[:])
                    nc.tensor.matmul(out=pzs[:], lhsT=z[:], rhs=ones[:],
                                     start=(half == 0), stop=(half == 1))
                zsum = sb.tile([EMB, 1], F32, tag="zsum")
                nc.vector.tensor_copy(zsum[:], pzs[:])
                nc.sync.dma_start(zs_in[:], zsum[:])

            nc.gpsimd.collective_compute(
                "AllReduce", OP.add, replica_groups=RG,
                ins=[zs_in.opt()], outs=[zs_out.opt()])

            # ---- phase 4: decoder ----------------------------------------
            rhs_zm = consts.tile([P, 2], F32)
            nc.vector.memset(rhs_zm[:], 0.0)
            nc.sync.dma_start(rhs_zm[0:EMB, 0:1], zs_out[:])
            nc.sync.dma_start(rhs_zm[EMB:2 * EMB, 1:2], zs_out[:])
            nc.scalar.mul(rhs_zm[:], rhs_zm[:], 1.0 / N)
            rhs_zmb = consts.tile([P, 2], BF16)
            nc.vector.tensor_copy(rhs_zmb[:], rhs_zm[:])

            with (
                tc.tile_pool(name="wd", bufs=3) as wdp,
                tc.tile_pool(name="dec", bufs=2) as decp,
                tc.tile_pool(name="ps4", bufs=2, space="PSUM") as ps4,
            ):
                pdec = None
                for g in range(WD_NGROUPS):
                    wd_sb = wdp.tile([P, WD_GROUP * P], BF16, tag="wd")
                    nc.scalar.dma_start(wd_sb[:], wd_d[g, :, :])
                    if g % 8 == 0:
                        pdec = ps4.tile([P, 512], F32, space="PSUM", tag="pdec")
                    for u in range(WD_GROUP):
                        t = g * WD_GROUP + u
                        u2 = t % 256
                        nc.tensor.matmul(
                            out=pdec[:, 2 * u2:2 * u2 + 2],
                            lhsT=wd_sb[:, u * P:(u + 1) * P], rhs=rhs_zmb[:],
                            start=True, stop=True)
                    if g % 8 == 7:
                        b = g // 8
                        bd_sb = decp.tile([P, 512], F32, tag="bd")
                        nc.scalar.dma_start(bd_sb[:], bd_d[b, :, :])
                        so = decp.tile([P, 512], F32, tag="so")
                        nc.vector.tensor_add(so[:], pdec[:], bd_sb[:])
                        nc.scalar.activation(so[:], so[:], AF.Sigmoid)
                        nc.sync.dma_start(out_d[b, :, :], so[:])

    nc.compile()
    _split_excess_waits(nc)
    return nc


_prog_cache = {}


def _get_program(T):
    if T not in _prog_cache:
        _prog_cache[T] = build_program(T)
    return _prog_cache[T]


def _rep(v, rows=P):
    v = np.asarray(v, np.float32).reshape(1, -1)
    return np.tile(v, (rows, 1)).copy()


def _wrap16(idx, ncall, per_call):
    """dma_gather index layout: per call, idx i sits at [i%16, i//16];
    the 16-partition block is replicated 8x down the partition axis."""
    w = idx.reshape(ncall, per_call // 16, 16).transpose(0, 2, 1)  # [c,16,s]
    w = np.tile(w, (1, 8, 1))                                      # [c,128,s]
    return np.ascontiguousarray(
        w.transpose(1, 0, 2).reshape(128, ncall * (per_call // 16)))


def prepare_inputs(inputs):
    """Host-side sharding: bucket edges by dst range, slice/pre-arrange Wd."""
    edge_index = np.asarray(inputs["edge_index"])
    x = np.asarray(inputs["x"], np.float32)
    eps = np.asarray(inputs["eps"], np.float32)
    W1 = np.asarray(inputs["W1"], np.float32)
    W2 = np.asarray(inputs["W2"], np.float32)
    Wmu = np.asarray(inputs["Wmu"], np.float32)
    Wlv = np.asarray(inputs["Wlv"], np.float32)
    Wd = np.asarray(inputs["Wd"], np.float32)
    bd = np.asarray(inputs["bd"], np.float32)

    loops = np.arange(N, dtype=np.int64)
    src = np.concatenate([edge_index[0].astype(np.int64), loops])
    dst = np.concatenate([edge_index[1].astype(np.int64), loops])
    core = dst // DPC
    counts = np.bincount(core, minlength=NCORES)
    T = int(np.ceil(counts.max() / P))
    T = ((T + GB - 1) // GB) * GB
    epad = T * P
    ncall = T // GB

    bf = ml_dtypes.bfloat16
    xT = np.ascontiguousarray(x.T).astype(bf)
    common = {
        "xT": xT, "W1": W1.astype(bf), "W2": W2.astype(bf),
        "Wmu": Wmu.astype(bf), "Wlv": Wlv.astype(bf),
        "asd1r": _rep(np.concatenate([
            np.asarray(inputs["att_src1"], np.float32).ravel(),
            np.asarray(inputs["att_dst1"], np.float32).ravel()])),
        "b1r": _rep(np.asarray(inputs["b1"], np.float32)),
        "as2r": _rep(np.asarray(inputs["att_src2"], np.float32)),
        "ad2r": _rep(np.asarray(inputs["att_dst2"], np.float32)),
        "b2r": _rep(np.asarray(inputs["b2"], np.float32)),
        "bmur": _rep(np.asarray(inputs["bmu"], np.float32)),
        "blvr": _rep(np.asarray(inputs["blv"], np.float32)),
    }

    in_maps = []
    for c in range(NCORES):
        m = dict(common)
        sel = core == c
        s_c, d_c = src[sel], dst[sel]
        k = len(s_c)
        es = np.zeros(epad, np.int64)
        es[:k] = s_c
        eg = np.zeros(epad, np.int64)
        eg[:k] = d_c
        el = np.full(epad, -1.0, np.float32)
        el[:k] = (d_c - c * DPC).astype(np.float32)
        m["esrc16"] = _wrap16(es.astype(np.int16), ncall, GB * P)
        m["edstg16"] = _wrap16(eg.astype(np.int16), ncall, GB * P)
        m["edstl"] = np.ascontiguousarray(el.reshape(T, P).T)
        m["epsl"] = np.ascontiguousarray(eps[c * DPC:(c + 1) * DPC])

        wslice = Wd[:, c * COLS:(c + 1) * COLS]
        X = wslice.reshape(EMB, 2048, 2, P)
        lhsT = np.empty((2048, P, P), np.float32)
        lhsT[:, 0:EMB, :] = X[:, :, 0, :].transpose(1, 0, 2)
        lhsT[:, EMB:P, :] = X[:, :, 1, :].transpose(1, 0, 2)
        m["wd"] = np.ascontiguousarray(
            lhsT.reshape(WD_NGROUPS, WD_GROUP, P, P)
                .transpose(0, 2, 1, 3).reshape(WD_NGROUPS, P, WD_GROUP * P)
                .astype(ml_dtypes.bfloat16))
        B = bd[c * COLS:(c + 1) * COLS].reshape(8, 256, 2, P)
        m["bd"] = np.ascontiguousarray(B.transpose(0, 3, 1, 2).reshape(8, P, 512))
        in_maps.append(m)
    return T, in_maps


def assemble_output(results):
    decoded = np.empty((N, N), np.float32)
    for c in range(NCORES):
        o = results[c]["out"]            # [8, 128, 512]
        F = o.reshape(8, P, 256, 2).transpose(0, 2, 3, 1).reshape(COLS)
        decoded[c * DPC:(c + 1) * DPC, :] = F.reshape(DPC, N)
    return decoded


def run(inputs, **run_kwargs):
    T, in_maps = prepare_inputs(inputs)
    nc = _get_program(T)
    res = run_bass_kernel_spmd(nc, in_maps, core_ids=list(range(NCORES)),
                               **run_kwargs)
    return assemble_output(res.results), res


def kernel(**inputs):
    out, _ = run(inputs)
    return out


# revision 20
# speedup vs baseline: 1.0227x; 1.0227x over previous
"""GAT-VGAE forward pass on 8 Trainium2 NeuronCores (Bass/Tile).

Strategy
--------
- Edges are bucketed by destination node range on the host: core c owns dst
  nodes [256c, 256c+256).  Segment-softmax over incoming edges never needs a
  segment max: the logits of this problem are bounded (|logit| < ~6), so
  exp(logit) is computed directly (softmax is shift-invariant).
- Per-edge gathers use batched SWDGE dma_gather (1280 indices per call) from
  "augmented" row tables (h | a_src | a_dst), edges landing on partitions.
- Segment sums (denominators + weighted message aggregation) are one-hot
  matmuls accumulated in PSUM: lhsT = onehot(dst_local) [128e x 256d],
  rhs = payload [128e x F].  One-hots are built once and reused by layer 2.
- Matmul operands are bf16 (PSUM accumulates fp32); attention arithmetic
  (logits, exp, normalization) stays fp32 on DVE/ACT.
- Layer-1 output (hidden) is transposed on-device and AllGathered (bf16) so
  each core can form lhsT tiles of hidden for the layer-2 matmul.
- z-mean is a ones-matmul partition reduction + tiny AllReduce.
- The huge decoder weight Wd [64, N*N] (1 GiB) is sharded column-wise:
  67 MB/core in bf16, pre-arranged on the host into [128,128] lhsT tiles
  packing two 128-column chunks along K (rhs = [[zm,0],[0,zm]]), so each
  matmul streams 32 KB of Wd and lands 256 outputs on 128 partitions.
  Decoder weight DMA rides the ACT HWDGE ring so it cannot head-of-line
  block the phase-critical loads on the sync ring.  Sigmoid is applied by
  ScalarE straight out of PSUM bank fills.
"""
import sys

sys.path.insert(0, '/opt/trn_rl_repo')

import ml_dtypes
import numpy as np

import bass_rust
import concourse.bass as bass
import concourse.bacc as bacc
import concourse.mybir as mybir
import concourse.tile as tile
from concourse import library_config
from concourse.bass_utils import run_bass_kernel_spmd
from concourse.masks import make_identity
from concourse.tile import add_dep_helper

F32 = mybir.dt.float32
BF16 = mybir.dt.bfloat16
I16 = mybir.dt.int16
AF = mybir.ActivationFunctionType
OP = mybir.AluOpType

N = 2048
F_IN = 256
C1 = 128
H = 4
HID = H * C1          # 512
EMB = 64
NCORES = 8
DPC = N // NCORES     # 256 dst nodes per core
COLS = N * N // NCORES  # 524288 decoder columns per core
NEG = 0.2
P = 128
H1ROW = 576           # h1(512) | a_src1(4) | a_dst1(4) | pad -> 2304B rows
H2ROW = 128           # h2(64) | a_src2(1) | a_dst2(1) | pad -> 512B rows
DROW = 64             # dst-table rows: 256B
GB = 8                # edge tiles per dma_gather call (1024 idxs; >1024 crashes SWDGE)
WD_GROUP = 32         # decoder lhsT tiles per DMA group
WD_NGROUPS = COLS // (256 * WD_GROUP)  # 64
RG = [list(range(NCORES))]

_MAX_WAITS = 1
_wait_ctr = [0]


def _split_excess_waits(nc):
    """This container's walrus accepts only one sync-wait per instruction.
    Hoist excess waits onto InstNoOps inserted just before, same engine."""
    for f in nc.m.functions:
        for blk in f.blocks:
            out = []
            changed = False
            for inst in blk.instructions:
                si = inst.sync_info
                waits = list(si.on_wait) if si is not None else []
                if len(waits) > _MAX_WAITS:
                    changed = True
                    extra, keep = waits[:-_MAX_WAITS], waits[-_MAX_WAITS:]
                    for i in range(0, len(extra), _MAX_WAITS):
                        nop = bass_rust.InstNoOp(
                            name=f"waitsplit-{_wait_ctr[0]}", ins=[], outs=[])
                        _wait_ctr[0] += 1
                        nop.engine = inst.engine
                        nop.sync_info = bass_rust.SyncInfo(
                            on_wait=extra[i:i + _MAX_WAITS], on_update=[])
                        out.append(nop)
                    inst.sync_info = bass_rust.SyncInfo(
                        on_wait=keep, on_update=list(si.on_update))
                out.append(inst)
            if changed:
                blk.instructions = out


def _leaky(nc, sb, x_ap, w):
    """leaky_relu(x) = max(x, NEG*x) on DVE (ACT Lrelu ignores alpha)."""
    t = sb.tile([P, w], F32)
    nc.vector.tensor_scalar_mul(t[:], x_ap, NEG)
    nc.vector.tensor_tensor(out=t[:], in0=t[:], in1=x_ap, op=OP.max)
    return t


def build_program(T):
    """T = number of 128-edge tiles per core (multiple of GB)."""
    assert T % GB == 0
    ncall = T // GB
    icols = GB * P // 16  # idx columns per gather call (64)
    nc = bacc.Bacc("TRN2", num_devices=NCORES)

    # ---- I/O -------------------------------------------------------------
    xT_d = nc.dram_tensor("xT", [F_IN, N], BF16, kind="ExternalInput")
    w1_d = nc.dram_tensor("W1", [F_IN, HID], BF16, kind="ExternalInput")
    w2_d = nc.dram_tensor("W2", [HID, EMB], BF16, kind="ExternalInput")
    wmu_d = nc.dram_tensor("Wmu", [EMB, EMB], BF16, kind="ExternalInput")
    wlv_d = nc.dram_tensor("Wlv", [EMB, EMB], BF16, kind="ExternalInput")
    asd1_d = nc.dram_tensor("asd1r", [P, 2 * HID], F32, kind="ExternalInput")
    b1_d = nc.dram_tensor("b1r", [P, HID], F32, kind="ExternalInput")
    as2_d = nc.dram_tensor("as2r", [P, EMB], F32, kind="ExternalInput")
    ad2_d = nc.dram_tensor("ad2r", [P, EMB], F32, kind="ExternalInput")
    b2_d = nc.dram_tensor("b2r", [P, EMB], F32, kind="ExternalInput")
    bmu_d = nc.dram_tensor("bmur", [P, EMB], F32, kind="ExternalInput")
    blv_d = nc.dram_tensor("blvr", [P, EMB], F32, kind="ExternalInput")
    eps_d = nc.dram_tensor("epsl", [DPC, EMB], F32, kind="ExternalInput")
    esrc16_d = nc.dram_tensor("esrc16", [P, ncall * icols], I16,
                              kind="ExternalInput")
    edstg16_d = nc.dram_tensor("edstg16", [P, ncall * icols], I16,
                               kind="ExternalInput")
    edstl_d = nc.dram_tensor("edstl", [P, T], F32, kind="ExternalInput")
    wd_d = nc.dram_tensor("wd", [WD_NGROUPS, P, WD_GROUP * P], BF16,
                          kind="ExternalInput")
    bd_d = nc.dram_tensor("bd", [8, P, 512], F32, kind="ExternalInput")
    out_d = nc.dram_tensor("out", [8, P, 512], F32, kind="ExternalOutput")

    # ---- internal DRAM gather tables -------------------------------------
    h1aug_d = nc.dram_tensor("h1aug", [N, H1ROW], F32, kind="Internal")
    daug1_d = nc.dram_tensor("daug1", [N, DROW], F32, kind="Internal")
    dlocal2_d = nc.dram_tensor("dlocal2", [DPC, 1], F32, kind="Internal")

    with tile.TileContext(nc) as tc:
        with (
            tc.tile_pool(name="consts", bufs=1) as consts,
            tc.tile_pool(name="dram", bufs=1, space="DRAM") as dram,
            tc.tile_pool(name="sb", bufs=3) as sb,
        ):
            # ---- constants ------------------------------------------------
            iota_i = consts.tile([P, 2 * P], mybir.dt.int32)
            iota_inst = nc.gpsimd.iota(iota_i[:], pattern=[[1, 2 * P]], base=0,
                                       channel_multiplier=0)
            iota_f = consts.tile([P, 2 * P], F32)
            nc.vector.tensor_copy(iota_f[:], iota_i[:])
            lib_inst = nc.gpsimd.load_library(library_config.mlp)
            add_dep_helper(lib_inst.ins, iota_inst.ins, sync=True,
                           reason="iota (standard lib) before mlp lib load")
            ident = consts.tile([P, P], F32)
            make_identity(nc, ident[:])
            ones = consts.tile([P, 1], F32)
            nc.vector.memset(ones[:], 1.0)

            xt_sb = [consts.tile([P, N], BF16, tag=f"xt{i}", name=f"xt{i}")
                     for i in range(2)]
            for i in range(2):
                nc.sync.dma_start(xt_sb[i][:], xT_d[i * P:(i + 1) * P, :])
            w1_sb = [consts.tile([P, HID], BF16, tag=f"w1{i}", name=f"w1s{i}")
                     for i in range(2)]
            for i in range(2):
                nc.sync.dma_start(w1_sb[i][:], w1_d[i * P:(i + 1) * P, :])
            w2_sb = [consts.tile([P, EMB], BF16, tag=f"w2{i}", name=f"w2s{i}")
                     for i in range(4)]
            for i in range(4):
                nc.sync.dma_start(w2_sb[i][:], w2_d[i * P:(i + 1) * P, :])
            wmu_sb = consts.tile([EMB, EMB], BF16)
            nc.sync.dma_start(wmu_sb[:], wmu_d[:])
            wlv_sb = consts.tile([EMB, EMB], BF16)
            nc.sync.dma_start(wlv_sb[:], wlv_d[:])
            asd1_sb = consts.tile([P, 2 * HID], F32)
            nc.scalar.dma_start(asd1_sb[:], asd1_d[:])
            b1_sb = consts.tile([P, HID], F32)
            nc.scalar.dma_start(b1_sb[:], b1_d[:])
            as2_sb = consts.tile([P, EMB], F32)
            nc.scalar.dma_start(as2_sb[:], as2_d[:])
            ad2_sb = consts.tile([P, EMB], F32)
            nc.scalar.dma_start(ad2_sb[:], ad2_d[:])
            b2_sb = consts.tile([P, EMB], F32)
            nc.scalar.dma_start(b2_sb[:], b2_d[:])
            bmu_sb = consts.tile([P, EMB], F32)
            nc.scalar.dma_start(bmu_sb[:], bmu_d[:])
            blv_sb = consts.tile([P, EMB], F32)
            nc.scalar.dma_start(blv_sb[:], blv_d[:])
            eps_sb = [consts.tile([P, EMB], F32, tag=f"eps{i}", name=f"epss{i}")
                      for i in range(2)]
            for i in range(2):
                nc.sync.dma_start(eps_sb[i][:], eps_d[i * P:(i + 1) * P, :])
            esrc16_sb = consts.tile([P, ncall * icols], I16)
            nc.sync.dma_start(esrc16_sb[:], esrc16_d[:])
            edstg16_sb = consts.tile([P, ncall * icols], I16)
            nc.sync.dma_start(edstg16_sb[:], edstg16_d[:])
            edstl_sb = consts.tile([P, T], F32)
            nc.sync.dma_start(edstl_sb[:], edstl_d[:])
            # one-hots built in phase 1, reused in phase 2b
            ohall = consts.tile([P, T * 2 * P], BF16)

            def gather(table, idx_sb, c, width, tag, bufs=2):
                g = sb.tile([P, GB, width], F32, tag=tag, name=f"{tag}{c}",
                            bufs=bufs)
                gi = nc.gpsimd.dma_gather(
                    g[:], table[:, :], idx_sb[:, c * icols:(c + 1) * icols],
                    GB * P, GB * P, width)
                add_dep_helper(gi.ins, lib_inst.ins, sync=True,
                               reason="dma_gather needs mlp library")
                return g

            # ---- phase 0: h1 = x @ W1, a_src1/a_dst1, build gather tables -
            with tc.tile_pool(name="ps0", bufs=2, space="PSUM") as ps0:
                for m in range(N // P):
                    ph1 = ps0.tile([P, HID], F32, space="PSUM")
                    for ck in range(2):
                        nc.tensor.matmul(
                            out=ph1[:], lhsT=xt_sb[ck][:, m * P:(m + 1) * P],
                            rhs=w1_sb[ck][:], start=(ck == 0), stop=(ck == 1))
                    aug = sb.tile([P, 520], F32, tag="h1aug", bufs=2)
                    nc.scalar.copy(aug[:, 0:HID], ph1[:])
                    tmp = sb.tile([P, 2 * HID], F32, tag="p0tmp", bufs=2)
                    nc.vector.tensor_tensor(
                        out=tmp[:].rearrange("p (s h c) -> p s h c", s=2, h=H),
                        in0=aug[:, 0:HID].rearrange(
                            "p (h c) -> p h c", h=H)[:, None, :, :]
                            .to_broadcast([P, 2, H, C1]),
                        in1=asd1_sb[:].rearrange("p (s h c) -> p s h c",
                                                 s=2, h=H),
                        op=OP.mult)
                    nc.vector.tensor_reduce(
                        out=aug[:, HID:520],
                        in_=tmp[:].rearrange("p (s h c) -> p (s h) c", s=2,
                                             h=H),
                        axis=mybir.AxisListType.X, op=OP.add)
                    nc.sync.dma_start(h1aug_d[m * P:(m + 1) * P, 0:520], aug[:])
                    nc.sync.dma_start(daug1_d[m * P:(m + 1) * P, 0:H],
                                      aug[:, HID + H:520])

            # ---- phase 1: layer-1 edge pass -------------------------------
            hidT_sb = [consts.tile([P, 2 * P], BF16, tag=f"hidT{i}",
                                   name=f"hidT{i}") for i in range(4)]
            with tc.tile_pool(name="ps1", bufs=1, space="PSUM") as ps1:
                pd1 = [ps1.tile([P, HID], F32, space="PSUM", tag=f"pd1{i}",
                                name=f"pd1{i}") for i in range(2)]
                pden = [ps1.tile([P, H], F32, space="PSUM", tag=f"pden{i}",
                                 name=f"pden{i}") for i in range(2)]
                for c in range(ncall):
                    ehg = gather(h1aug_d, esrc16_sb, c, H1ROW, "ehg")
                    eadg = gather(daug1_d, edstg16_sb, c, DROW, "eadg")
                    # batched per-call edge math: one DVE/ACT op per stage
                    lg = sb.tile([P, GB, H], F32, tag="lg1")
                    nc.vector.tensor_tensor(
                        out=lg[:], in0=ehg[:, :, HID:HID + H],
                        in1=eadg[:, :, 0:H], op=OP.add)
                    lr = sb.tile([P, GB, H], F32, tag="lr1")
                    nc.vector.tensor_scalar_mul(lr[:], lg[:], NEG)
                    nc.vector.tensor_tensor(out=lr[:], in0=lr[:], in1=lg[:],
                                            op=OP.max)
                    v = sb.tile([P, GB, H], F32, tag="v1")
                    nc.scalar.activation(v[:], lr[:], AF.Exp)
                    vb = sb.tile([P, GB, H], BF16, tag="vb1")
                    nc.vector.tensor_copy(vb[:], v[:])
                    pay = sb.tile([P, GB, HID], BF16, tag="pay1", bufs=2)
                    nc.vector.tensor_tensor(
                        out=pay[:].rearrange("p g (h c) -> p g h c", h=H),
                        in0=ehg[:, :, 0:HID].rearrange(
                            "p g (h c) -> p g h c", h=H),
                        in1=v[:, :, :, None].to_broadcast([P, GB, H, C1]),
                        op=OP.mult)
                    ohc = ohall[:, c * GB * 2 * P:(c + 1) * GB * 2 * P]
                    nc.vector.tensor_tensor(
                        out=ohc.rearrange("p (g j) -> p g j", g=GB),
                        in0=edstl_sb[:, c * GB:(c + 1) * GB, None]
                            .to_broadcast([P, GB, 2 * P]),
                        in1=iota_f[:, None, :].to_broadcast([P, GB, 2 * P]),
                        op=OP.is_equal)
                    for u in range(GB):
                        t = c * GB + u
                        oh = ohall[:, t * 2 * P:(t + 1) * 2 * P]
                        st, sp = (t == 0), (t == T - 1)
                        for half in range(2):
                            ohh = oh[:, half * P:(half + 1) * P]
                            nc.tensor.matmul(out=pd1[half][:], lhsT=ohh,
                                             rhs=pay[:, u, :], start=st, stop=sp)
                            nc.tensor.matmul(out=pden[half][:], lhsT=ohh,
                                             rhs=vb[:, u, :], start=st, stop=sp)

                # normalize + bias + relu + transpose
                recip = sb.tile([P, 2 * H], F32, tag="recip1")
                for half in range(2):
                    nc.vector.tensor_scalar_add(
                        recip[:, half * H:(half + 1) * H], pden[half][:], 1e-16)
                nc.vector.reciprocal(recip[:], recip[:])
                with tc.tile_pool(name="psT", bufs=2, space="PSUM") as psT:
                    for half in range(2):
                        agg = sb.tile([P, HID], F32, tag="agg1", bufs=2)
                        nc.scalar.copy(agg[:], pd1[half][:])
                        hid = sb.tile([P, HID], F32, tag="hid", bufs=2)
                        nc.vector.tensor_tensor(
                            out=hid[:].rearrange("p (h c) -> p h c", h=H),
                            in0=agg[:].rearrange("p (h c) -> p h c", h=H),
                            in1=recip[:, half * H:(half + 1) * H]
                                .to_broadcast([P, H, C1]),
                            op=OP.mult)
                        nc.vector.tensor_add(hid[:], hid[:], b1_sb[:])
                        nc.scalar.activation(hid[:], hid[:], AF.Relu)
                        for ck in range(4):
                            pt = psT.tile([P, P], F32, space="PSUM", tag="pt")
                            nc.tensor.transpose(
                                out=pt[:], in_=hid[:, ck * P:(ck + 1) * P],
                                identity=ident[:])
                            nc.vector.tensor_copy(
                                hidT_sb[ck][:, half * P:(half + 1) * P], pt[:])

            # ---- phase 2a: local h2 from local hidden, AllGather the table
            h2loc = dram.tile([DPC, H2ROW], F32)
            h2full = dram.tile([N, H2ROW], F32)
            with (
                tc.tile_pool(name="ps2a", bufs=1, space="PSUM") as ps2a,
                tc.tile_pool(name="ps2t", bufs=2, space="PSUM") as ps2t,
            ):
                ph2t = ps2a.tile([EMB, 2 * P], F32, space="PSUM", tag="ph2t")
                for ck in range(4):
                    nc.tensor.matmul(out=ph2t[:], lhsT=w2_sb[ck][:],
                                     rhs=hidT_sb[ck][:],
                                     start=(ck == 0), stop=(ck == 3))
                h2t_sb = sb.tile([EMB, 2 * P], F32, tag="h2ts")
                nc.vector.tensor_copy(h2t_sb[:], ph2t[:])
                for half in range(2):
                    pt = ps2t.tile([P, EMB], F32, space="PSUM", tag="p2t")
                    nc.tensor.transpose(
                        out=pt[:], in_=h2t_sb[:, half * P:(half + 1) * P],
                        identity=ident[0:EMB, 0:EMB])
                    aug2 = sb.tile([P, EMB + 2], F32, tag="h2aug")
                    nc.scalar.copy(aug2[:, 0:EMB], pt[:])
                    tmp2 = sb.tile([P, EMB], F32, tag="p2tmp")
                    nc.vector.tensor_tensor(out=tmp2[:], in0=pt[:],
                                            in1=as2_sb[:], op=OP.mult)
                    nc.vector.tensor_reduce(out=aug2[:, EMB:EMB + 1],
                                            in_=tmp2[:],
                                            axis=mybir.AxisListType.X,
                                            op=OP.add)
                    nc.vector.tensor_tensor(out=tmp2[:], in0=pt[:],
                                            in1=ad2_sb[:], op=OP.mult)
                    nc.vector.tensor_reduce(out=aug2[:, EMB + 1:EMB + 2],
                                            in_=tmp2[:],
                                            axis=mybir.AxisListType.X,
                                            op=OP.add)
                    nc.sync.dma_start(
                        h2loc[half * P:(half + 1) * P, 0:EMB + 2], aug2[:])
                    nc.sync.dma_start(dlocal2_d[half * P:(half + 1) * P, :],
                                      aug2[:, EMB + 1:EMB + 2])
            nc.gpsimd.collective_compute(
                "AllGather", OP.bypass, replica_groups=RG,
                ins=[h2loc.opt()], outs=[h2full.opt()])

            # replicate local a_dst2 across partitions for the DVE expansion
            adst2_rep = consts.tile([P, DPC], F32)
            nc.sync.dma_start(
                adst2_rep[:],
                dlocal2_d[:, :].rearrange("a b -> b a").to_broadcast(
                    [P, DPC]))

            # ---- phase 2b: layer-2 edge pass ------------------------------
            embT_sb = consts.tile([EMB, 2 * P], BF16)
            with tc.tile_pool(name="ps2b", bufs=1, space="PSUM") as ps2b:
                pd2 = [ps2b.tile([P, EMB], F32, space="PSUM", tag=f"pd2{i}",
                                 name=f"pd2{i}") for i in range(2)]
                pden2 = [ps2b.tile([P, 1], F32, space="PSUM", tag=f"pden2{i}",
                                   name=f"pden2{i}") for i in range(2)]
                for c in range(ncall):
                    eh2g = gather(h2full, esrc16_sb, c, H2ROW, "eh2g")
                    ohc = ohall[:, c * GB * 2 * P:(c + 1) * GB * 2 * P]
                    adx = sb.tile([P, GB, 2 * P], F32, tag="adx", bufs=2)
                    nc.vector.tensor_tensor(
                        out=adx[:],
                        in0=ohc.rearrange("p (g j) -> p g j", g=GB),
                        in1=adst2_rep[:, None, :].to_broadcast([P, GB, 2 * P]),
                        op=OP.mult)
                    ead2 = sb.tile([P, GB, 1], F32, tag="ead2")
                    nc.vector.tensor_reduce(out=ead2[:], in_=adx[:],
                                            axis=mybir.AxisListType.X,
                                            op=OP.add)
                    lg2 = sb.tile([P, GB, 1], F32, tag="lg2")
                    nc.vector.tensor_tensor(
                        out=lg2[:], in0=eh2g[:, :, EMB:EMB + 1],
                        in1=ead2[:], op=OP.add)
                    lr2 = sb.tile([P, GB, 1], F32, tag="lr2")
                    nc.vector.tensor_scalar_mul(lr2[:], lg2[:], NEG)
                    nc.vector.tensor_tensor(out=lr2[:], in0=lr2[:], in1=lg2[:],
                                            op=OP.max)
                    v2 = sb.tile([P, GB, 1], F32, tag="v2")
                    nc.scalar.activation(v2[:], lr2[:], AF.Exp)
                    v2b = sb.tile([P, GB, 1], BF16, tag="v2b")
                    nc.vector.tensor_copy(v2b[:], v2[:])
                    pay2 = sb.tile([P, GB, EMB], BF16, tag="pay2")
                    nc.vector.tensor_tensor(
                        out=pay2[:], in0=eh2g[:, :, 0:EMB],
                        in1=v2[:].to_broadcast([P, GB, EMB]), op=OP.mult)
                    for u in range(GB):
                        t = c * GB + u
                        oh = ohall[:, t * 2 * P:(t + 1) * 2 * P]
                        st, sp = (t == 0), (t == T - 1)
                        for half in range(2):
                            ohh = oh[:, half * P:(half + 1) * P]
                            nc.tensor.matmul(out=pd2[half][:], lhsT=ohh,
                                             rhs=pay2[:, u, :], start=st, stop=sp)
                            nc.tensor.matmul(out=pden2[half][:], lhsT=ohh,
                                             rhs=v2b[:, u, :], start=st, stop=sp)

                recip2 = sb.tile([P, 2], F32, tag="recip2")
                for half in range(2):
                    nc.vector.tensor_scalar_add(
                        recip2[:, half:half + 1], pden2[half][:], 1e-16)
                nc.vector.reciprocal(recip2[:], recip2[:])
                with tc.tile_pool(name="psT2", bufs=2, space="PSUM") as psT2:
                    for half in range(2):
                        agg2 = sb.tile([P, EMB], F32, tag="agg2", bufs=2)
                        nc.scalar.copy(agg2[:], pd2[half][:])
                        emb = sb.tile([P, EMB], F32, tag="emb")
                        nc.vector.tensor_tensor(
                            out=emb[:], in0=agg2[:],
                            in1=recip2[:, half:half + 1].to_broadcast([P, EMB]),
                            op=OP.mult)
                        nc.vector.tensor_add(emb[:], emb[:], b2_sb[:])
                        pt2 = psT2.tile([EMB, P], F32, space="PSUM", tag="pt2")
                        nc.tensor.transpose(out=pt2[:], in_=emb[:],
                                            identity=ident[:])
                        nc.vector.tensor_copy(
                            embT_sb[:, half * P:(half + 1) * P], pt2[:])

            # ---- phase 3: mu / logvar / z / z-sum -------------------------
            zs_in = dram.tile([EMB, 1], F32)
            zs_out = dram.tile([EMB, 1], F32)
            with tc.tile_pool(name="ps3", bufs=1, space="PSUM") as ps3:
                pzs = ps3.tile([EMB, 1], F32, space="PSUM", tag="pzs")
                for half in range(2):
                    lhs = embT_sb[:, half * P:(half + 1) * P]
                    pmu = ps3.tile([P, EMB], F32, space="PSUM",
                                   tag=f"pmu{half}", name=f"pmu{half}")
                    nc.tensor.matmul(out=pmu[:], lhsT=lhs, rhs=wmu_sb[:],
                                     start=True, stop=True)
                    plv = ps3.tile([P, EMB], F32, space="PSUM",
                                   tag=f"plv{half}", name=f"plv{half}")
                    nc.tensor.matmul(out=plv[:], lhsT=lhs, rhs=wlv_sb[:],
                                     start=True, stop=True)
                    elv = sb.tile([P, EMB], F32, tag="elv")
                    nc.vector.tensor_add(elv[:], plv[:], blv_sb[:])
                    nc.scalar.activation(elv[:], elv[:], AF.Exp, scale=0.5)
                    z = sb.tile([P, EMB], F32, tag="z")
                    nc.vector.tensor_tensor(out=z[:], in0=elv[:],
                                            in1=eps_sb[half][:], op=OP.mult)
                    nc.vector.tensor_add(z[:], z[:], bmu_sb[:])
                    nc.vector.tensor_add(z[:], z[:], pmu[:])
                    nc.tensor.matmul(out=pzs[:], lhsT=z[:], rhs=ones[:],
                                     start=(half == 0), stop=(half == 1))
                zsum = sb.tile([EMB, 1], F32, tag="zsum")
                nc.vector.tensor_copy(zsum[:], pzs[:])
                nc.sync.dma_start(zs_in[:], zsum[:])

            nc.gpsimd.collective_compute(
                "AllReduce", OP.add, replica_groups=RG,
                ins=[zs_in.opt()], outs=[zs_out.opt()])

            # ---- phase 4: decoder ----------------------------------------
            rhs_zm = consts.tile([P, 2], F32)
            nc.vector.memset(rhs_zm[:], 0.0)
            nc.sync.dma_start(rhs_zm[0:EMB, 0:1], zs_out[:])
            nc.sync.dma_start(rhs_zm[EMB:2 * EMB, 1:2], zs_out[:])
            nc.scalar.mul(rhs_zm[:], rhs_zm[:], 1.0 / N)
            rhs_zmb = consts.tile([P, 2], BF16)
            nc.vector.tensor_copy(rhs_zmb[:], rhs_zm[:])

            with (
                tc.tile_pool(name="wd", bufs=3) as wdp,
                tc.tile_pool(name="dec", bufs=2) as decp,
                tc.tile_pool(name="ps4", bufs=2, space="PSUM") as ps4,
            ):
                pdec = None
                for g in range(WD_NGROUPS):
                    wd_sb = wdp.tile([P, WD_GROUP * P], BF16, tag="wd")
                    nc.scalar.dma_start(wd_sb[:], wd_d[g, :, :])
                    if g % 8 == 0:
                        pdec = ps4.tile([P, 512], F32, space="PSUM", tag="pdec")
                    for u in range(WD_GROUP):
                        t = g * WD_GROUP + u
                        u2 = t % 256
                        nc.tensor.matmul(
                            out=pdec[:, 2 * u2:2 * u2 + 2],
                            lhsT=wd_sb[:, u * P:(u + 1) * P], rhs=rhs_zmb[:],
                            start=True, stop=True)
                    if g % 8 == 7:
                        b = g // 8
                        bd_sb = decp.tile([P, 512], F32, tag="bd")
                        nc.scalar.dma_start(bd_sb[:], bd_d[b, :, :])
                        so = decp.tile([P, 512], F32, tag="so")
                        nc.vector.tensor_add(so[:], pdec[:], bd_sb[:])
                        nc.scalar.activation(so[:], so[:], AF.Sigmoid)
                        nc.sync.dma_start(out_d[b, :, :], so[:])

    nc.compile()
    _split_excess_waits(nc)
    return nc


_prog_cache = {}


def _get_program(T):
    if T not in _prog_cache:
        _prog_cache[T] = build_program(T)
    return _prog_cache[T]


def _rep(v, rows=P):
    v = np.asarray(v, np.float32).reshape(1, -1)
    return np.tile(v, (rows, 1)).copy()


def _wrap16(idx, ncall, per_call):
    """dma_gather index layout: per call, idx i sits at [i%16, i//16];
    the 16-partition block is replicated 8x down the partition axis."""
    w = idx.reshape(ncall, per_call // 16, 16).transpose(0, 2, 1)  # [c,16,s]
    w = np.tile(w, (1, 8, 1))                                      # [c,128,s]
    return np.ascontiguousarray(
        w.transpose(1, 0, 2).reshape(128, ncall * (per_call // 16)))


def prepare_inputs(inputs):
    """Host-side sharding: bucket edges by dst range, slice/pre-arrange Wd."""
    edge_index = np.asarray(inputs["edge_index"])
    x = np.asarray(inputs["x"], np.float32)
    eps = np.asarray(inputs["eps"], np.float32)
    W1 = np.asarray(inputs["W1"], np.float32)
    W2 = np.asarray(inputs["W2"], np.float32)
    Wmu = np.asarray(inputs["Wmu"], np.float32)
    Wlv = np.asarray(inputs["Wlv"], np.float32)
    Wd = np.asarray(inputs["Wd"], np.float32)
    bd = np.asarray(inputs["bd"], np.float32)

    loops = np.arange(N, dtype=np.int64)
    src = np.concatenate([edge_index[0].astype(np.int64), loops])
    dst = np.concatenate([edge_index[1].astype(np.int64), loops])
    core = dst // DPC
    counts = np.bincount(core, minlength=NCORES)
    T = int(np.ceil(counts.max() / P))
    T = ((T + GB - 1) // GB) * GB
    epad = T * P
    ncall = T // GB

    bf = ml_dtypes.bfloat16
    xT = np.ascontiguousarray(x.T).astype(bf)
    common = {
        "xT": xT, "W1": W1.astype(bf), "W2": W2.astype(bf),
        "Wmu": Wmu.astype(bf), "Wlv": Wlv.astype(bf),
        "asd1r": _rep(np.concatenate([
            np.asarray(inputs["att_src1"], np.float32).ravel(),
            np.asarray(inputs["att_dst1"], np.float32).ravel()])),
        "b1r": _rep(np.asarray(inputs["b1"], np.float32)),
        "as2r": _rep(np.asarray(inputs["att_src2"], np.float32)),
        "ad2r": _rep(np.asarray(inputs["att_dst2"], np.float32)),
        "b2r": _rep(np.asarray(inputs["b2"], np.float32)),
        "bmur": _rep(np.asarray(inputs["bmu"], np.float32)),
        "blvr": _rep(np.asarray(inputs["blv"], np.float32)),
    }

    in_maps = []
    for c in range(NCORES):
        m = dict(common)
        sel = core == c
        s_c, d_c = src[sel], dst[sel]
        k = len(s_c)
        es = np.zeros(epad, np.int64)
        es[:k] = s_c
        eg = np.zeros(epad, np.int64)
        eg[:k] = d_c
        el = np.full(epad, -1.0, np.float32)
        el[:k] = (d_c - c * DPC).astype(np.float32)
        m["esrc16"] = _wrap16(es.astype(np.int16), ncall, GB * P)
        m["edstg16"] = _wrap16(eg.astype(np.int16), ncall, GB * P)
        m["edstl"] = np.ascontiguousarray(el.reshape(T, P).T)
        m["epsl"] = np.ascontiguousarray(eps[c * DPC:(c + 1) * DPC])

        wslice = Wd[:, c * COLS:(c + 1) * COLS]
        X = wslice.reshape(EMB, 2048, 2, P)
        lhsT = np.empty((2048, P, P), np.float32)
        lhsT[:, 0:EMB, :] = X[:, :, 0, :].transpose(1, 0, 2)
        lhsT[:, EMB:P, :] = X[:, :, 1, :].transpose(1, 0, 2)
        m["wd"] = np.ascontiguousarray(
            lhsT.reshape(WD_NGROUPS, WD_GROUP, P, P)
                .transpose(0, 2, 1, 3).reshape(WD_NGROUPS, P, WD_GROUP * P)
                .astype(ml_dtypes.bfloat16))
        B = bd[c * COLS:(c + 1) * COLS].reshape(8, 256, 2, P)
        m["bd"] = np.ascontiguousarray(B.transpose(0, 3, 1, 2).reshape(8, P, 512))
        in_maps.append(m)
    return T, in_maps


def assemble_output(results):
    decoded = np.empty((N, N), np.float32)
    for c in range(NCORES):
        o = results[c]["out"]            # [8, 128, 512]
        F = o.reshape(8, P, 256, 2).transpose(0, 2, 3, 1).reshape(COLS)
        decoded[c * DPC:(c + 1) * DPC, :] = F.reshape(DPC, N)
    return decoded


def run(inputs, **run_kwargs):
    T, in_maps = prepare_inputs(inputs)
    nc = _get_program(T)
    last_err = None
    for _attempt in range(3):
        try:
            res = run_bass_kernel_spmd(nc, in_maps,
                                       core_ids=list(range(NCORES)),
                                       **run_kwargs)
            return assemble_output(res.results), res
        except Exception as e:  # transient NRT device errors
            last_err = e
    raise last_err


def kernel(**inputs):
    out, _ = run(inputs)
    return out
